# revision 46
# baseline (speedup 1.0000x reference)
"""Multi-head attention (B=4, S=2048, D=512, H=8, Dh=64) on 8 trn2 NeuronCores.

Sharding: core c = b*2 + hg handles batch b and head-group hg (4 heads = 2
pairs p, 2 heads f per pair). Host pre-transposes x and casts x/W to bf16,
so all matmuls run at the 1-cycle/row bf16 rate (fp32r measures ~2 cyc/row
on HW) and no on-device transposes are needed.

Per core: Q^T/K^T projections (head dims on partitions, bias on DVE), V in
natural layout with a ones column (softmax denominators fall out of the AV
matmul). The attention is a single flat stream of 3-slot windows across
all 8 (qb, p) blocks: scores^T = K Q^T as row-group-concurrent
64-contraction pairs into a 3-bank PSUM window, exp on ACT over the window
(N=1536) into bf16, AV matmuls one window behind so the PE never queues
behind an ACT wait — including across block boundaries. V-projection tiles
are injected just-in-time into early windows. Softmax normalize: oT
evicted to SBUF (frees PSUM fast), one batched reciprocal per query block,
DMA partition-broadcast, multiply on DVE(f0)/Pool(f1). The row-parallel
out-projection of block qb is delayed a full block so its normalize chain
is never on the PE critical path. Head-group partials summed on host
(plus bv@Wo + bo).
"""
import numpy as np

import concourse.bass as bass
import concourse.mybir as mybir
import concourse.tile as tile

F32 = mybir.dt.float32
BF16 = mybir.dt.bfloat16
I16 = mybir.dt.int16

# Schraudolph-style exp for the DVE-offloaded windows: bf16 bits of exp(s/8)
# ~= int16(round(s * (log2e/8 * 128) + (127*128 - C))). C tuned for minimax
# relative error (~2% rms); softmax normalization cancels the mean bias.
SCHR_A = float(np.log2(np.e) / 8.0 * 128.0)
SCHR_B = float(127.0 * 128.0 - 4.55)
# fraction pattern: every 4th window's exp runs on DVE instead of ACT
SCHR_PERIOD = 4

B, S, D_IN, H, D_HEAD = 4, 2048, 512, 8, 64
HG = 2                      # head groups (tensor-parallel shards)
H_LOC = H // HG             # 4 heads per core
DO = H_LOC * D_HEAD         # 256 projected dims per core
N_CORES = B * HG
P = 128
KC = D_IN // P              # 4 contraction chunks
ST = S // P                 # 16 key tiles
QB = 4                      # query blocks
QBS = S // QB               # 512
NP = 2                      # head pairs per core
WIN = 3                     # exp window: 3 (kt,f) slots of [128,512]
NSLOT = ST * 2              # 32 slots per (qb, p)
NW = (NSLOT + WIN - 1) // WIN  # 11 windows per block (last has 2 slots)
NB = QB * NP                # 8 blocks

# ---------------------------------------------------------------------------
# walrus in this container rejects >1 sync-wait per instruction: split the
# extras onto single-wait NOPs inserted before the instruction (same engine).
_ENGINES_WITH_NOP = {
    mybir.EngineType.PE,
    mybir.EngineType.Activation,
    mybir.EngineType.DVE,
    mybir.EngineType.Pool,
    mybir.EngineType.SP,
}


def _split_multi_waits(nc, max_waits=1):
    cnt = 0
    for fn in nc.m.functions:
        for blk in fn.blocks:
            out = []
            changed = False
            for inst in blk.instructions:
                si = getattr(inst, "sync_info", None)
                waits = list(si.on_wait) if si is not None else []
                if len(waits) > max_waits and inst.engine in _ENGINES_WITH_NOP:
                    changed = True
                    for w in waits[:-max_waits]:
                        cnt += 1
                        out.append(
                            mybir.InstNoOp(
                                name=f"I-wsplit-{cnt}",
                                engine=inst.engine,
                                ins=[],
                                outs=[],
                                sync_info=mybir.SyncInfo(on_wait=[w], on_update=[]),
                            )
                        )
                    inst.sync_info = mybir.SyncInfo(
                        on_wait=waits[-max_waits:], on_update=list(si.on_update)
                    )
                out.append(inst)
            if changed:
                blk.instructions = out


# ---------------------------------------------------------------------------


def build_program(loop_iters=None):
    nc = bass.Bass()

    xqT = nc.declare_dram_parameter("xqT", [D_IN, S], BF16, isOutput=False)
    xkT = nc.declare_dram_parameter("xkT", [D_IN, S], BF16, isOutput=False)
    xvT = nc.declare_dram_parameter("xvT", [D_IN, S], BF16, isOutput=False)
    wq = nc.declare_dram_parameter("wq", [D_IN, DO], BF16, isOutput=False)
    wk = nc.declare_dram_parameter("wk", [D_IN, DO], BF16, isOutput=False)
    wv = nc.declare_dram_parameter("wv", [D_IN, DO], BF16, isOutput=False)
    # host pre-permutes wo to [dh, f, pair, dout] so it lands partition-major
    wo = nc.declare_dram_parameter("wo", [D_HEAD, 2 * NP * D_HEAD], BF16, isOutput=False)
    bqp = nc.declare_dram_parameter("bq", [DO], F32, isOutput=False)
    bkp = nc.declare_dram_parameter("bk", [DO], F32, isOutput=False)
    out = nc.declare_dram_parameter("out", [S, D_HEAD], F32, isOutput=True)

    with tile.TileContext(nc) as tc:
        with (
            tc.tile_pool(name="cst", bufs=1) as cst,
            tc.tile_pool(name="exp", bufs=4) as exp_pool,
            tc.tile_pool(name="small", bufs=2) as small,
            tc.tile_pool(name="outst", bufs=2) as outst,
            tc.tile_pool(name="sc_ps", bufs=2, space="PSUM") as sc_ps,
            tc.tile_pool(name="oT_ps", bufs=1, space="PSUM") as oT_ps,
        ):
            # ---------------- persistent SBUF ----------------
            from concourse.masks import make_identity

            ident = cst.tile([P, P], F32)
            make_identity(nc, ident[:])
            bqs = cst.tile([1, DO], F32)
            bks = cst.tile([1, DO], F32)
            bq_sb = cst.tile([P, NP], F32)
            bk_sb = cst.tile([P, NP], F32)
            w_sb = {
                name: cst.tile([P, KC, DO], BF16, name=f"{name}_sb")
                for name in ("wq", "wk", "wv")
            }
            # wo with head-dim on partitions 0-63 (matches attn slices)
            wo_sb = cst.tile([D_HEAD, 2, NP, D_HEAD], BF16)
            xq_sb = cst.tile([P, KC, S], BF16, name="xq_sb")
            xk_sb = cst.tile([P, KC, S], BF16, name="xk_sb")
            xv_sb = cst.tile([P, KC, S], BF16, name="xv_sb")
            qtp = cst.tile([P, NP, S], BF16, name="qtp")
            ktp = cst.tile([P, NP, S], BF16, name="ktp")
            # V: [keys-in-tile, kt, pair, f, dh+ones]
            v_sb = cst.tile([P, ST, NP, 2, D_HEAD + 1], BF16, name="v_sb")
            nc.vector.memset(v_sb[:, :, :, :, D_HEAD : D_HEAD + 1], 1.0)
            # unnormalized per-head attention outputs: [dh, f, pair, q]
            attn = cst.tile([D_HEAD, 2, NP, S], BF16, name="attn")

            from contextlib import ExitStack as _ES
            _loop = _ES()
            if loop_iters is not None:
                _loop.enter_context(tc.For_i(0, loop_iters, 1))

            # ---------------- input DMAs on the two HWDGE queues ----------
            # (gpsimd/SWDGE DMAs fail codegen inside hardware loops.)
            # sync: xq chunks (Q proj is first PE work), then xv 0-1.
            # scalar: small weights first, then xk, wv, xv 2-3.
            for c in range(KC):
                nc.sync.dma_start(xq_sb[:, c, :], xqT[c * P : (c + 1) * P, :])
            nc.scalar.dma_start(bqs[:], bqp[None, :])
            nc.scalar.dma_start(bks[:], bkp[None, :])
            nc.scalar.dma_start(w_sb["wq"][:], wq.rearrange("(c p) o -> p c o", p=P))
            nc.scalar.dma_start(w_sb["wk"][:], wk.rearrange("(c p) o -> p c o", p=P))
            for c in range(KC):
                nc.scalar.dma_start(xk_sb[:, c, :], xkT[c * P : (c + 1) * P, :])
            nc.scalar.dma_start(w_sb["wv"][:], wv.rearrange("(c p) o -> p c o", p=P))
            for c in range(2):
                nc.sync.dma_start(xv_sb[:, c, :], xvT[c * P : (c + 1) * P, :])
            for c in range(2, KC):
                nc.scalar.dma_start(xv_sb[:, c, :], xvT[c * P : (c + 1) * P, :])
            nc.sync.dma_start(
                wo_sb[:], wo.rearrange("c (f a o) -> c f a o", f=2, a=NP)
            )

            # biases arrive as [1, 256] rows; transposed onto partitions via
            # PE. Emitted after the first Q matmul group so the (later) bias
            # DMA never blocks the head of the PE queue.
            def emit_bias_transposes():
                bps = sc_ps.tile([P, WIN, QBS], F32, tag="sc")
                for mc in range(NP):
                    nc.tensor.transpose(
                        bps[:, 0, mc : mc + 1],
                        bqs[0:1, mc * P : (mc + 1) * P],
                        ident[0:1, 0:1],
                    )
                    nc.tensor.transpose(
                        bps[:, 0, NP + mc : NP + mc + 1],
                        bks[0:1, mc * P : (mc + 1) * P],
                        ident[0:1, 0:1],
                    )
                nc.vector.tensor_copy(out=bq_sb[:], in_=bps[:, 0, 0:NP])
                nc.vector.tensor_copy(out=bk_sb[:], in_=bps[:, 0, NP : 2 * NP])

            # ---------------- Q / K projections ----------------
            def proj_qk(which, x_sb, dst, bias, after_first=None):
                for p in range(NP):
                    for qc in range(QB):
                        ps = sc_ps.tile([P, WIN, QBS], F32, tag="sc")
                        for kc in range(KC):
                            nc.tensor.matmul(
                                ps[:, 0, :],
                                w_sb[which][:, kc, p * P : (p + 1) * P],
                                x_sb[:, kc, qc * QBS : (qc + 1) * QBS],
                                start=(kc == 0),
                                stop=(kc == KC - 1),
                            )
                        if after_first is not None:
                            after_first()
                            after_first = None
                        nc.vector.tensor_scalar(
                            out=dst[:, p, qc * QBS : (qc + 1) * QBS],
                            in0=ps[:, 0, :],
                            scalar1=bias[:, p : p + 1],
                            scalar2=None,
                            op0=mybir.AluOpType.add,
                        )

            proj_qk("wq", xq_sb, qtp, bq_sb, after_first=emit_bias_transposes)
            proj_qk("wk", xk_sb, ktp, bk_sb)

            def emit_vproj(st):
                ps = sc_ps.tile([P, WIN, QBS], F32, tag="sc")
                for kc in range(KC):
                    nc.tensor.matmul(
                        ps[:, 0, 0:DO],
                        xv_sb[:, kc, st * P : (st + 1) * P],
                        w_sb["wv"][:, kc, :],
                        start=(kc == 0),
                        stop=(kc == KC - 1),
                    )
                nc.vector.tensor_copy(
                    out=v_sb[:, st, :, :, 0:D_HEAD],
                    in_=ps[:, 0, 0:DO].rearrange("p (a f d) -> p a f d", a=NP, f=2),
                )

            emit_vproj(0)
            emit_vproj(1)

            # ---------------- attention: flat window stream ----------------
            scale = float(1.0 / np.sqrt(D_HEAD))
            TOT = NB * NW
            blocks = [(qb, p) for qb in range(QB) for p in range(NP)]

            def base(bi):
                return bi * NW

            st_state = [dict(sc=[None] * NW, ex=[None] * NW, oT=None) for _ in blocks]
            dens = {}        # qb -> den tile [4 (p,f), QBS] fp32

            def emit_scores(j):
                bi, w = j // NW, j % NW
                qb, p = blocks[bi]
                n = min(WIN, NSLOT - w * WIN)
                sct = sc_ps.tile([P, WIN, QBS], F32, tag="sc")
                st_state[bi]["sc"][w] = (sct, n)
                for i in range(n):
                    s = w * WIN + i
                    kt, f = s // 2, s % 2
                    nc.tensor.matmul(
                        sct[:, i, :],
                        ktp[f * 64 : (f + 1) * 64, p, kt * P : (kt + 1) * P],
                        qtp[f * 64 : (f + 1) * 64, p, qb * QBS : (qb + 1) * QBS],
                        start=True,
                        stop=True,
                    )

            def emit_exp(j):
                bi, w = j // NW, j % NW
                sct, n = st_state[bi]["sc"][w]
                ext = exp_pool.tile([P, WIN, QBS], BF16, tag="exp")
                st_state[bi]["ex"][w] = (ext, n)
                if j % SCHR_PERIOD == SCHR_PERIOD - 1:
                    # offloaded window: Schraudolph bf16-bit exp on DVE
                    nc.vector.tensor_scalar(
                        out=ext[:, 0:n, :].bitcast(I16),
                        in0=sct[:, 0:n, :],
                        scalar1=SCHR_A,
                        scalar2=SCHR_B,
                        op0=mybir.AluOpType.mult,
                        op1=mybir.AluOpType.add,
                    )
                else:
                    nc.scalar.activation(
                        ext[:, 0:n, :],
                        sct[:, 0:n, :],
                        mybir.ActivationFunctionType.Exp,
                        scale=scale,
                    )

            def emit_av(j):
                bi, w = j // NW, j % NW
                qb, p = blocks[bi]
                stt = st_state[bi]
                if stt["oT"] is None:
                    stt["oT"] = oT_ps.tile(
                        [D_HEAD + 1, 2, QBS], F32, tag="oT", name=f"oT{bi}"
                    )
                oT = stt["oT"]
                ext, n = stt["ex"][w]
                for i in range(n):
                    s = w * WIN + i
                    kt, f = s // 2, s % 2
                    nc.tensor.matmul(
                        oT[:, f, :],
                        v_sb[:, kt, p, f, :],
                        ext[:, i, :],
                        start=(kt == 0),
                        stop=(kt == ST - 1),
                    )

            def emit_evict(bi):
                # evict oT: unnormalized per-head output to attn (bf16) and
                # the ones-row denominators to den (fp32, partition-shifted)
                qb, p = blocks[bi]
                oT = st_state[bi]["oT"]
                # denominator row first: it heads the reciprocal chain
                if p == 0:
                    dens[qb] = small.tile([4, QBS], F32, tag="den", name=f"den{qb}")
                dblk = small.tile([1, 2, QBS], F32, tag="dblk", name=f"dblk{bi}")
                nc.vector.tensor_copy(out=dblk[:], in_=oT[D_HEAD : D_HEAD + 1, :, :])
                nc.scalar.dma_start(dens[qb][2 * p : 2 * p + 2, :], dblk[:])
                nc.vector.tensor_copy(
                    out=attn[:, :, p, qb * QBS : (qb + 1) * QBS],
                    in_=oT[0:D_HEAD, :, :],
                )

            def emit_qbfinish(qb):
                """Transpose denominators to query-partitions, one cheap
                reciprocal, per-head out-projection, per-partition-scaled
                combine, one batched store."""
                ps = sc_ps.tile([P, WIN, QBS], F32, tag="sc")
                den = dens[qb]
                for i in range(QBS // P):
                    nc.tensor.transpose(
                        ps[:, 0, 4 * i : 4 * i + 4],
                        den[:, i * P : (i + 1) * P],
                        ident[0:4, 0:4],
                    )
                recT = small.tile([P, 16], F32, tag="recT")
                nc.vector.reciprocal(recT[:], ps[:, 0, 0:16])
                o_st = outst.tile([P, QBS // P, D_HEAD], F32, tag="ost")

                def sl(i):
                    return ps[:, 1 + i // 2, (i % 2) * DO : (i % 2) * DO + DO]

                # all 16 matmuls first, then all combines — the PE never
                # queues behind the DVE combine chain
                for i in range(QBS // P):
                    qt = qb * (QBS // P) + i
                    for j in range(4):
                        p_, f_ = j // 2, j % 2
                        nc.tensor.matmul(
                            sl(i)[:, j * D_HEAD : (j + 1) * D_HEAD],
                            attn[:, f_, p_, qt * P : (qt + 1) * P],
                            wo_sb[:, f_, p_, :],
                            start=True,
                            stop=True,
                        )
                for i in range(QBS // P):
                    nc.vector.tensor_scalar(
                        out=o_st[:, i, :],
                        in0=sl(i)[:, 0:D_HEAD],
                        scalar1=recT[:, 4 * i : 4 * i + 1],
                        scalar2=None,
                        op0=mybir.AluOpType.mult,
                    )
                    for j in range(1, 4):
                        nc.vector.scalar_tensor_tensor(
                            out=o_st[:, i, :],
                            in0=sl(i)[:, j * D_HEAD : (j + 1) * D_HEAD],
                            scalar=recT[:, 4 * i + j : 4 * i + j + 1],
                            in1=o_st[:, i, :],
                            op0=mybir.AluOpType.mult,
                            op1=mybir.AluOpType.add,
                        )
                nc.sync.dma_start(
                    out[qb * QBS : (qb + 1) * QBS, :].rearrange(
                        "(t p) d -> p t d", p=P
                    ),
                    o_st[:],
                )

            # hooks keyed by global stream index, run after that index's AV
            hooks = {}
            for st in range(2, ST):
                # v_sb[st] is consumed by AV of window (2*st)//3, which is
                # emitted at stream index (2*st)//3 + 1 — inject the V
                # projection a couple of windows ahead of that.
                hooks.setdefault(max(0, (2 * st) // 3 - 2), []).append(
                    (emit_vproj, (st,))
                )
            for bi in range(1, NB):
                # evict/den for the previous block right after its last AV is
                # emitted (at stream index base(bi)) and BEFORE block bi's
                # first AV reuses the single-buffer oT psum tile.
                hooks.setdefault(base(bi), []).append((emit_evict, (bi - 1,)))
            for qb in range(QB):
                bi_p1 = qb * NP + 1
                if bi_p1 + 1 < NB:
                    hooks.setdefault(base(bi_p1 + 1) + 2, []).append(
                        (emit_qbfinish, (qb,))
                    )

            emit_scores(0)
            for j in range(TOT):
                if j + 1 < TOT:
                    emit_scores(j + 1)
                emit_exp(j)
                if j > 0:
                    emit_av(j - 1)
                for fn, args in hooks.pop(j, ()):
                    fn(*args)
            emit_av(TOT - 1)
            # tail: last block evict + last qb finish
            emit_evict(NB - 1)
            emit_qbfinish(QB - 1)

            _loop.close()

    _split_multi_waits(nc)
    return nc


class _Runner:
    """Compile once; keep a jitted shard_map executable around."""

    def __init__(self, nc=None):
        import jax
        from jax.experimental.shard_map import shard_map
        from jax.sharding import Mesh, NamedSharding, PartitionSpec
        from concourse import bass2jax

        bass2jax.install_neuronx_cc_hook()
        if nc is None:
            nc = build_program()
        self.nc = nc
        self.jax = jax

        partition_name = (
            nc.partition_id_tensor.name if nc.partition_id_tensor else None
        )
        in_names, out_names, out_avals, zero_outs = [], [], [], []
        for alloc in nc.m.functions[0].allocations:
            if not isinstance(alloc, mybir.MemoryLocationSet):
                continue
            name = alloc.memorylocations[0].name
            if alloc.kind == "ExternalInput":
                if name != partition_name:
                    in_names.append(name)
            elif alloc.kind == "ExternalOutput":
                out_names.append(name)
                shape = tuple(alloc.tensor_shape)
                dtype = mybir.dt.np(alloc.dtype)
                out_avals.append(jax.core.ShapedArray(shape, dtype))
                zero_outs.append(np.zeros(shape, dtype))
        self.in_names = list(in_names)
        self.out_names = out_names
        self.out_avals = out_avals
        self.zero_outs = zero_outs
        n_params = len(in_names)
        n_outs = len(out_avals)
        all_in_names = in_names + out_names
        if partition_name is not None:
            all_in_names.append(partition_name)
        donate = tuple(range(n_params, n_params + n_outs))

        def _body(*args):
            operands = list(args)
            if partition_name is not None:
                operands.append(bass2jax.partition_id_tensor())
            outs = bass2jax._bass_exec_p.bind(
                *operands,
                out_avals=tuple(out_avals),
                in_names=tuple(all_in_names),
                out_names=tuple(out_names),
                lowering_input_output_aliases=(),
                sim_require_finite=True,
                sim_require_nnan=True,
                nc=nc,
            )
            return tuple(outs)

        devices = jax.devices()[:N_CORES]
        mesh = Mesh(np.asarray(devices), ("core",))
        self.mesh = mesh
        self.sharding = NamedSharding(mesh, PartitionSpec("core"))
        in_specs = (PartitionSpec("core"),) * (n_params + n_outs)
        out_specs = (PartitionSpec("core"),) * len(out_names)
        self.fn = jax.jit(
            shard_map(
                _body, mesh=mesh, in_specs=in_specs,
                out_specs=out_specs, check_rep=False,
            ),
            donate_argnums=donate,
            keep_unused=True,
        )

    def put_inputs(self, in_maps):
        concat = [
            np.concatenate([np.asarray(in_maps[c][n]) for c in range(N_CORES)], axis=0)
            for n in self.in_names
        ]
        return [self.jax.device_put(a, self.sharding) for a in concat]

    def make_zeros(self):
        return [
            self.jax.device_put(
                np.zeros((N_CORES * z.shape[0], *z.shape[1:]), z.dtype), self.sharding
            )
            for z in self.zero_outs
        ]

    def run(self, in_dev):
        out_arrs = self.fn(*in_dev, *self.make_zeros())
        return [
            {
                n: np.asarray(out_arrs[i]).reshape(N_CORES, *self.out_avals[i].shape)[c]
                for i, n in enumerate(self.out_names)
            }
            for c in range(N_CORES)
        ]


_RUNNER = None


def _get_runner():
    global _RUNNER
    if _RUNNER is None:
        _RUNNER = _Runner()
    return _RUNNER


def _make_in_maps(query, key, value, Wq, Wk, Wv, Wo, bq, bk):
    import ml_dtypes

    bf = ml_dtypes.bfloat16
    in_maps = []
    xT = {}
    for b in range(B):
        xT[b] = (
            np.ascontiguousarray(query[b].T).astype(bf),
            np.ascontiguousarray(key[b].T).astype(bf),
            np.ascontiguousarray(value[b].T).astype(bf),
        )
    for c in range(N_CORES):
        b, hg = divmod(c, HG)
        sl = slice(hg * DO, (hg + 1) * DO)
        in_maps.append(
            {
                "xqT": xT[b][0],
                "xkT": xT[b][1],
                "xvT": xT[b][2],
                "wq": np.ascontiguousarray(Wq[:, sl]).astype(bf),
                "wk": np.ascontiguousarray(Wk[:, sl]).astype(bf),
                "wv": np.ascontiguousarray(Wv[:, sl]).astype(bf),
                "wo": np.ascontiguousarray(
                    Wo[sl, :]
                    .reshape(NP, 2, D_HEAD, D_HEAD)
                    .transpose(2, 1, 0, 3)
                    .reshape(D_HEAD, 2 * NP * D_HEAD)
                ).astype(bf),
                "bq": np.ascontiguousarray(bq[sl]),
                "bk": np.ascontiguousarray(bk[sl]),
            }
        )
    return in_maps


def kernel(query, key, value, Wq, bq, Wk, bk, Wv, bv, Wo, bo):
    query = np.ascontiguousarray(np.asarray(query, dtype=np.float32))
    key = np.ascontiguousarray(np.asarray(key, dtype=np.float32))
    value = np.ascontiguousarray(np.asarray(value, dtype=np.float32))
    Wq = np.asarray(Wq, dtype=np.float32)
    Wk = np.asarray(Wk, dtype=np.float32)
    Wv = np.asarray(Wv, dtype=np.float32)
    Wo = np.asarray(Wo, dtype=np.float32)
    bq = np.asarray(bq, dtype=np.float32)
    bk = np.asarray(bk, dtype=np.float32)
    bv = np.asarray(bv, dtype=np.float32)
    bo = np.asarray(bo, dtype=np.float32)

    r = _get_runner()
    in_dev = r.put_inputs(_make_in_maps(query, key, value, Wq, Wk, Wv, Wo, bq, bk))
    results = r.run(in_dev)

    out = np.zeros((B, S, D_HEAD), dtype=np.float32)
    for c in range(N_CORES):
        b = c // HG
        out[b] += results[c]["out"]
    out += bv @ Wo + bo
    return out


def bench(query, key, value, Wq, bq, Wk, bk, Wv, bv, Wo, bo, iters=20):
    """Steady-state per-iteration wall time of the device execution."""
    import time

    r = _get_runner()
    in_dev = r.put_inputs(
        _make_in_maps(
            np.asarray(query, np.float32), np.asarray(key, np.float32),
            np.asarray(value, np.float32), np.asarray(Wq, np.float32),
            np.asarray(Wk, np.float32), np.asarray(Wv, np.float32),
            np.asarray(Wo, np.float32), np.asarray(bq, np.float32),
            np.asarray(bk, np.float32),
        )
    )
    outs = r.fn(*in_dev, *r.make_zeros())
    self_jax = r.jax
    self_jax.block_until_ready(outs)
    zeros = [r.make_zeros() for _ in range(iters)]
    t0 = time.monotonic()
    last = None
    for i in range(iters):
        last = r.fn(*in_dev, *zeros[i])
    self_jax.block_until_ready(last)
    t1 = time.monotonic()
    return (t1 - t0) / iters


# revision 47
# speedup vs baseline: 1.1808x; 1.1808x over previous
"""Multi-head attention (B=4, S=2048, D=512, H=8, Dh=64) on 8 trn2 NeuronCores.

Sharding: core c = b*2 + hg handles batch b and head-group hg (4 heads = 2
pairs p, 2 heads f per pair). Host pre-transposes x and casts x/W to bf16,
so all matmuls run at the 1-cycle/row bf16 rate (fp32r measures ~2 cyc/row
on HW) and no on-device transposes are needed.

Per core: Q^T/K^T projections (head dims on partitions, bias on DVE), V in
natural layout with a ones column (softmax denominators fall out of the AV
matmul). The attention is a single flat stream of 3-slot windows across
all 8 (qb, p) blocks: scores^T = K Q^T as row-group-concurrent
64-contraction pairs into a 3-bank PSUM window, exp on ACT over the window
(N=1536) into bf16, AV matmuls one window behind so the PE never queues
behind an ACT wait — including across block boundaries. V-projection tiles
are injected just-in-time into early windows. Softmax normalize: oT
evicted to SBUF (frees PSUM fast), one batched reciprocal per query block,
DMA partition-broadcast, multiply on DVE(f0)/Pool(f1). The row-parallel
out-projection of block qb is delayed a full block so its normalize chain
is never on the PE critical path. Head-group partials summed on host
(plus bv@Wo + bo).
"""
import numpy as np

import concourse.bass as bass
import concourse.mybir as mybir
import concourse.tile as tile

F32 = mybir.dt.float32
BF16 = mybir.dt.bfloat16
I16 = mybir.dt.int16

# Schraudolph-style exp for the DVE-offloaded windows: bf16 bits of exp(s/8)
# ~= int16(round(s * (log2e/8 * 128) + (127*128 - C))). C tuned for minimax
# relative error (~2% rms); softmax normalization cancels the mean bias.
SCHR_A = float(np.log2(np.e) / 8.0 * 128.0)
SCHR_B = float(127.0 * 128.0 - 4.55)
# fraction pattern: every 4th window's exp runs on DVE instead of ACT
SCHR_PERIOD = 4

B, S, D_IN, H, D_HEAD = 4, 2048, 512, 8, 64
HG = 2                      # head groups (tensor-parallel shards)
H_LOC = H // HG             # 4 heads per core
DO = H_LOC * D_HEAD         # 256 projected dims per core
N_CORES = B * HG
P = 128
KC = D_IN // P              # 4 contraction chunks
ST = S // P                 # 16 key tiles
QB = 4                      # query blocks
QBS = S // QB               # 512
NP = 2                      # head pairs per core
WIN = 3                     # exp window: 3 (kt,f) slots of [128,512]
NSLOT = ST * 2              # 32 slots per (qb, p)
NW = (NSLOT + WIN - 1) // WIN  # 11 windows per block (last has 2 slots)
NB = QB * NP                # 8 blocks

# ---------------------------------------------------------------------------
# walrus in this container rejects >1 sync-wait per instruction: split the
# extras onto single-wait NOPs inserted before the instruction (same engine).
_ENGINES_WITH_NOP = {
    mybir.EngineType.PE,
    mybir.EngineType.Activation,
    mybir.EngineType.DVE,
    mybir.EngineType.Pool,
    mybir.EngineType.SP,
}


def _split_multi_waits(nc, max_waits=1):
    cnt = 0
    for fn in nc.m.functions:
        for blk in fn.blocks:
            out = []
            changed = False
            for inst in blk.instructions:
                si = getattr(inst, "sync_info", None)
                waits = list(si.on_wait) if si is not None else []
                if len(waits) > max_waits and inst.engine in _ENGINES_WITH_NOP:
                    changed = True
                    for w in waits[:-max_waits]:
                        cnt += 1
                        out.append(
                            mybir.InstNoOp(
                                name=f"I-wsplit-{cnt}",
                                engine=inst.engine,
                                ins=[],
                                outs=[],
                                sync_info=mybir.SyncInfo(on_wait=[w], on_update=[]),
                            )
                        )
                    inst.sync_info = mybir.SyncInfo(
                        on_wait=waits[-max_waits:], on_update=list(si.on_update)
                    )
                out.append(inst)
            if changed:
                blk.instructions = out


# ---------------------------------------------------------------------------


def build_program(loop_iters=None):
    nc = bass.Bass()

    xqT = nc.declare_dram_parameter("xqT", [D_IN, S], BF16, isOutput=False)
    xkT = nc.declare_dram_parameter("xkT", [D_IN, S], BF16, isOutput=False)
    xvT = nc.declare_dram_parameter("xvT", [D_IN, S], BF16, isOutput=False)
    wq = nc.declare_dram_parameter("wq", [D_IN, DO], BF16, isOutput=False)
    wk = nc.declare_dram_parameter("wk", [D_IN, DO], BF16, isOutput=False)
    wv = nc.declare_dram_parameter("wv", [D_IN, DO], BF16, isOutput=False)
    # host pre-permutes wo to [dh, f, pair, dout] so it lands partition-major
    wo = nc.declare_dram_parameter("wo", [D_HEAD, 2 * NP * D_HEAD], BF16, isOutput=False)
    bqp = nc.declare_dram_parameter("bq", [DO], F32, isOutput=False)
    bkp = nc.declare_dram_parameter("bk", [DO], F32, isOutput=False)
    out = nc.declare_dram_parameter("out", [S, D_HEAD], F32, isOutput=True)

    with tile.TileContext(nc) as tc:
        with (
            tc.tile_pool(name="cst", bufs=1) as cst,
            tc.tile_pool(name="exp", bufs=4) as exp_pool,
            tc.tile_pool(name="small", bufs=2) as small,
            tc.tile_pool(name="outst", bufs=2) as outst,
            tc.tile_pool(name="sc_ps", bufs=2, space="PSUM") as sc_ps,
            tc.tile_pool(name="oT_ps", bufs=1, space="PSUM") as oT_ps,
        ):
            # ---------------- persistent SBUF ----------------
            from concourse.masks import make_identity

            ident = cst.tile([P, P], F32)
            make_identity(nc, ident[:])
            bqs = cst.tile([1, DO], F32)
            bks = cst.tile([1, DO], F32)
            bq_sb = cst.tile([P, NP], F32)
            bk_sb = cst.tile([P, NP], F32)
            w_sb = {
                name: cst.tile([P, KC, DO], BF16, name=f"{name}_sb")
                for name in ("wq", "wk", "wv")
            }
            # wo with head-dim on partitions 0-63 (matches attn slices)
            wo_sb = cst.tile([D_HEAD, 2, NP, D_HEAD], BF16)
            xq_sb = cst.tile([P, KC, S], BF16, name="xq_sb")
            xk_sb = cst.tile([P, KC, S], BF16, name="xk_sb")
            xv_sb = cst.tile([P, KC, S], BF16, name="xv_sb")
            qtp = cst.tile([P, NP, S], BF16, name="qtp")
            ktp = cst.tile([P, NP, S], BF16, name="ktp")
            # V: [keys-in-tile, kt, pair, f, dh+ones]
            v_sb = cst.tile([P, ST, NP, 2, D_HEAD + 1], BF16, name="v_sb")
            nc.vector.memset(v_sb[:, :, :, :, D_HEAD : D_HEAD + 1], 1.0)
            # unnormalized per-head attention outputs: [dh, f, pair, q]
            attn = cst.tile([D_HEAD, 2, NP, S], BF16, name="attn")

            from contextlib import ExitStack as _ES
            _loop = _ES()
            if loop_iters is not None:
                _loop.enter_context(tc.For_i(0, loop_iters, 1))

            # ---------------- input DMAs on the two HWDGE queues ----------
            # (gpsimd/SWDGE DMAs fail codegen inside hardware loops.)
            # sync: xq chunks (Q proj is first PE work), then xv 0-1.
            # scalar: small weights first, then xk, wv, xv 2-3.
            for c in range(KC):
                nc.sync.dma_start(xq_sb[:, c, :], xqT[c * P : (c + 1) * P, :])
            nc.scalar.dma_start(bqs[:], bqp[None, :])
            nc.scalar.dma_start(bks[:], bkp[None, :])
            nc.scalar.dma_start(w_sb["wq"][:], wq.rearrange("(c p) o -> p c o", p=P))
            nc.scalar.dma_start(w_sb["wk"][:], wk.rearrange("(c p) o -> p c o", p=P))
            for c in range(KC):
                nc.scalar.dma_start(xk_sb[:, c, :], xkT[c * P : (c + 1) * P, :])
            nc.scalar.dma_start(w_sb["wv"][:], wv.rearrange("(c p) o -> p c o", p=P))
            for c in range(2):
                nc.sync.dma_start(xv_sb[:, c, :], xvT[c * P : (c + 1) * P, :])
            for c in range(2, KC):
                nc.scalar.dma_start(xv_sb[:, c, :], xvT[c * P : (c + 1) * P, :])
            nc.sync.dma_start(
                wo_sb[:], wo.rearrange("c (f a o) -> c f a o", f=2, a=NP)
            )

            # biases arrive as [1, 256] rows; transposed onto partitions via
            # PE. Emitted after the first Q matmul group so the (later) bias
            # DMA never blocks the head of the PE queue.
            def emit_bias_transposes():
                bps = sc_ps.tile([P, WIN, QBS], F32, tag="sc")
                for mc in range(NP):
                    nc.tensor.transpose(
                        bps[:, 0, mc : mc + 1],
                        bqs[0:1, mc * P : (mc + 1) * P],
                        ident[0:1, 0:1],
                    )
                    nc.tensor.transpose(
                        bps[:, 0, NP + mc : NP + mc + 1],
                        bks[0:1, mc * P : (mc + 1) * P],
                        ident[0:1, 0:1],
                    )
                nc.vector.tensor_copy(out=bq_sb[:], in_=bps[:, 0, 0:NP])
                nc.vector.tensor_copy(out=bk_sb[:], in_=bps[:, 0, NP : 2 * NP])

            # ---------------- Q / K projections ----------------
            def proj_qk(which, x_sb, dst, bias, after_first=None):
                for p in range(NP):
                    for qc in range(QB):
                        ps = sc_ps.tile([P, WIN, QBS], F32, tag="sc")
                        for kc in range(KC):
                            nc.tensor.matmul(
                                ps[:, 0, :],
                                w_sb[which][:, kc, p * P : (p + 1) * P],
                                x_sb[:, kc, qc * QBS : (qc + 1) * QBS],
                                start=(kc == 0),
                                stop=(kc == KC - 1),
                            )
                        if after_first is not None:
                            after_first()
                            after_first = None
                        nc.vector.tensor_scalar(
                            out=dst[:, p, qc * QBS : (qc + 1) * QBS],
                            in0=ps[:, 0, :],
                            scalar1=bias[:, p : p + 1],
                            scalar2=None,
                            op0=mybir.AluOpType.add,
                        )

            proj_qk("wq", xq_sb, qtp, bq_sb, after_first=emit_bias_transposes)
            proj_qk("wk", xk_sb, ktp, bk_sb)

            def emit_vproj(st):
                ps = sc_ps.tile([P, WIN, QBS], F32, tag="sc")
                for kc in range(KC):
                    nc.tensor.matmul(
                        ps[:, 0, 0:DO],
                        xv_sb[:, kc, st * P : (st + 1) * P],
                        w_sb["wv"][:, kc, :],
                        start=(kc == 0),
                        stop=(kc == KC - 1),
                    )
                nc.vector.tensor_copy(
                    out=v_sb[:, st, :, :, 0:D_HEAD],
                    in_=ps[:, 0, 0:DO].rearrange("p (a f d) -> p a f d", a=NP, f=2),
                )

            emit_vproj(0)
            emit_vproj(1)

            # ---------------- attention: flat window stream ----------------
            scale = float(1.0 / np.sqrt(D_HEAD))
            TOT = NB * NW
            blocks = [(qb, p) for qb in range(QB) for p in range(NP)]

            def base(bi):
                return bi * NW

            st_state = [dict(sc=[None] * NW, ex=[None] * NW, oT=None) for _ in blocks]
            dens = {}        # qb -> den tile [4 (p,f), QBS] fp32

            def emit_scores(j):
                bi, w = j // NW, j % NW
                qb, p = blocks[bi]
                n = min(WIN, NSLOT - w * WIN)
                sct = sc_ps.tile([P, WIN, QBS], F32, tag="sc")
                st_state[bi]["sc"][w] = (sct, n)
                for i in range(n):
                    s = w * WIN + i
                    kt, f = s // 2, s % 2
                    nc.tensor.matmul(
                        sct[:, i, :],
                        ktp[f * 64 : (f + 1) * 64, p, kt * P : (kt + 1) * P],
                        qtp[f * 64 : (f + 1) * 64, p, qb * QBS : (qb + 1) * QBS],
                        start=True,
                        stop=True,
                    )

            def emit_exp(j):
                bi, w = j // NW, j % NW
                sct, n = st_state[bi]["sc"][w]
                ext = exp_pool.tile([P, WIN, QBS], BF16, tag="exp")
                st_state[bi]["ex"][w] = (ext, n)
                if j % SCHR_PERIOD == SCHR_PERIOD - 1:
                    # offloaded window: Schraudolph bf16-bit exp on DVE
                    nc.vector.tensor_scalar(
                        out=ext[:, 0:n, :].bitcast(I16),
                        in0=sct[:, 0:n, :],
                        scalar1=SCHR_A,
                        scalar2=SCHR_B,
                        op0=mybir.AluOpType.mult,
                        op1=mybir.AluOpType.add,
                    )
                else:
                    nc.scalar.activation(
                        ext[:, 0:n, :],
                        sct[:, 0:n, :],
                        mybir.ActivationFunctionType.Exp,
                        scale=scale,
                    )

            def emit_av(j):
                bi, w = j // NW, j % NW
                qb, p = blocks[bi]
                stt = st_state[bi]
                if stt["oT"] is None:
                    stt["oT"] = oT_ps.tile(
                        [D_HEAD + 1, 2, QBS], F32, tag="oT", name=f"oT{bi}"
                    )
                oT = stt["oT"]
                ext, n = stt["ex"][w]
                for i in range(n):
                    s = w * WIN + i
                    kt, f = s // 2, s % 2
                    nc.tensor.matmul(
                        oT[:, f, :],
                        v_sb[:, kt, p, f, :],
                        ext[:, i, :],
                        start=(kt == 0),
                        stop=(kt == ST - 1),
                    )

            def emit_evict(bi):
                # evict oT: unnormalized per-head output to attn (bf16) and
                # the ones-row denominators to den (fp32, partition-shifted)
                qb, p = blocks[bi]
                oT = st_state[bi]["oT"]
                # denominator row first: it heads the reciprocal chain
                if p == 0:
                    dens[qb] = small.tile([4, QBS], F32, tag="den", name=f"den{qb}")
                dblk = small.tile([1, 2, QBS], F32, tag="dblk", name=f"dblk{bi}")
                nc.vector.tensor_copy(out=dblk[:], in_=oT[D_HEAD : D_HEAD + 1, :, :])
                nc.scalar.dma_start(dens[qb][2 * p : 2 * p + 2, :], dblk[:])
                nc.vector.tensor_copy(
                    out=attn[:, :, p, qb * QBS : (qb + 1) * QBS],
                    in_=oT[0:D_HEAD, :, :],
                )

            def emit_qbfinish(qb):
                """Transpose denominators to query-partitions, one cheap
                reciprocal, per-head out-projection, per-partition-scaled
                combine, one batched store."""
                ps = sc_ps.tile([P, WIN, QBS], F32, tag="sc")
                den = dens[qb]
                for i in range(QBS // P):
                    nc.tensor.transpose(
                        ps[:, 0, 4 * i : 4 * i + 4],
                        den[:, i * P : (i + 1) * P],
                        ident[0:4, 0:4],
                    )
                recT = small.tile([P, 16], F32, tag="recT")
                nc.vector.reciprocal(recT[:], ps[:, 0, 0:16])
                o_st = outst.tile([P, QBS // P, D_HEAD], F32, tag="ost")

                def sl(i):
                    return ps[:, 1 + i // 2, (i % 2) * DO : (i % 2) * DO + DO]

                # all 16 matmuls first, then all combines — the PE never
                # queues behind the DVE combine chain
                for i in range(QBS // P):
                    qt = qb * (QBS // P) + i
                    for j in range(4):
                        p_, f_ = j // 2, j % 2
                        nc.tensor.matmul(
                            sl(i)[:, j * D_HEAD : (j + 1) * D_HEAD],
                            attn[:, f_, p_, qt * P : (qt + 1) * P],
                            wo_sb[:, f_, p_, :],
                            start=True,
                            stop=True,
                        )
                for i in range(QBS // P):
                    nc.vector.tensor_scalar(
                        out=o_st[:, i, :],
                        in0=sl(i)[:, 0:D_HEAD],
                        scalar1=recT[:, 4 * i : 4 * i + 1],
                        scalar2=None,
                        op0=mybir.AluOpType.mult,
                    )
                    for j in range(1, 4):
                        nc.vector.scalar_tensor_tensor(
                            out=o_st[:, i, :],
                            in0=sl(i)[:, j * D_HEAD : (j + 1) * D_HEAD],
                            scalar=recT[:, 4 * i + j : 4 * i + j + 1],
                            in1=o_st[:, i, :],
                            op0=mybir.AluOpType.mult,
                            op1=mybir.AluOpType.add,
                        )
                nc.sync.dma_start(
                    out[qb * QBS : (qb + 1) * QBS, :].rearrange(
                        "(t p) d -> p t d", p=P
                    ),
                    o_st[:],
                )

            # hooks keyed by global stream index, run after that index's AV
            hooks = {}
            for st in range(2, ST):
                # v_sb[st] is consumed by AV of window (2*st)//3, which is
                # emitted at stream index (2*st)//3 + 2 — inject the V
                # projection a couple of windows ahead of that.
                hooks.setdefault(max(0, (2 * st) // 3 - 2), []).append(
                    (emit_vproj, (st,))
                )
            for bi in range(1, NB):
                # evict/den for the previous block right after its last AV is
                # emitted (at stream index base(bi)+1 with the 2-window AV
                # lag) and BEFORE block bi's first AV reuses the
                # single-buffer oT psum tile (emitted at base(bi)+2).
                hooks.setdefault(base(bi) + 1, []).append((emit_evict, (bi - 1,)))
            for qb in range(QB):
                bi_p1 = qb * NP + 1
                if bi_p1 + 1 < NB:
                    hooks.setdefault(base(bi_p1 + 1) + 3, []).append(
                        (emit_qbfinish, (qb,))
                    )

            # AV runs two windows behind exp so neither ACT nor DVE exp
            # latency ever blocks the in-order PE queue.
            emit_scores(0)
            for j in range(TOT):
                if j + 1 < TOT:
                    emit_scores(j + 1)
                emit_exp(j)
                if j >= 2:
                    emit_av(j - 2)
                for fn, args in hooks.pop(j, ()):
                    fn(*args)
            emit_av(TOT - 2)
            emit_av(TOT - 1)
            # tail: last block evict + last qb finish
            emit_evict(NB - 1)
            emit_qbfinish(QB - 1)

            _loop.close()

    _split_multi_waits(nc)
    return nc


class _Runner:
    """Compile once; keep a jitted shard_map executable around."""

    def __init__(self, nc=None):
        import jax
        from jax.experimental.shard_map import shard_map
        from jax.sharding import Mesh, NamedSharding, PartitionSpec
        from concourse import bass2jax

        bass2jax.install_neuronx_cc_hook()
        if nc is None:
            nc = build_program()
        self.nc = nc
        self.jax = jax

        partition_name = (
            nc.partition_id_tensor.name if nc.partition_id_tensor else None
        )
        in_names, out_names, out_avals, zero_outs = [], [], [], []
        for alloc in nc.m.functions[0].allocations:
            if not isinstance(alloc, mybir.MemoryLocationSet):
                continue
            name = alloc.memorylocations[0].name
            if alloc.kind == "ExternalInput":
                if name != partition_name:
                    in_names.append(name)
            elif alloc.kind == "ExternalOutput":
                out_names.append(name)
                shape = tuple(alloc.tensor_shape)
                dtype = mybir.dt.np(alloc.dtype)
                out_avals.append(jax.core.ShapedArray(shape, dtype))
                zero_outs.append(np.zeros(shape, dtype))
        self.in_names = list(in_names)
        self.out_names = out_names
        self.out_avals = out_avals
        self.zero_outs = zero_outs
        n_params = len(in_names)
        n_outs = len(out_avals)
        all_in_names = in_names + out_names
        if partition_name is not None:
            all_in_names.append(partition_name)
        donate = tuple(range(n_params, n_params + n_outs))

        def _body(*args):
            operands = list(args)
            if partition_name is not None:
                operands.append(bass2jax.partition_id_tensor())
            outs = bass2jax._bass_exec_p.bind(
                *operands,
                out_avals=tuple(out_avals),
                in_names=tuple(all_in_names),
                out_names=tuple(out_names),
                lowering_input_output_aliases=(),
                sim_require_finite=True,
                sim_require_nnan=True,
                nc=nc,
            )
            return tuple(outs)

        devices = jax.devices()[:N_CORES]
        mesh = Mesh(np.asarray(devices), ("core",))
        self.mesh = mesh
        self.sharding = NamedSharding(mesh, PartitionSpec("core"))
        in_specs = (PartitionSpec("core"),) * (n_params + n_outs)
        out_specs = (PartitionSpec("core"),) * len(out_names)
        self.fn = jax.jit(
            shard_map(
                _body, mesh=mesh, in_specs=in_specs,
                out_specs=out_specs, check_rep=False,
            ),
            donate_argnums=donate,
            keep_unused=True,
        )

    def put_inputs(self, in_maps):
        concat = [
            np.concatenate([np.asarray(in_maps[c][n]) for c in range(N_CORES)], axis=0)
            for n in self.in_names
        ]
        return [self.jax.device_put(a, self.sharding) for a in concat]

    def make_zeros(self):
        return [
            self.jax.device_put(
                np.zeros((N_CORES * z.shape[0], *z.shape[1:]), z.dtype), self.sharding
            )
            for z in self.zero_outs
        ]

    def run(self, in_dev):
        out_arrs = self.fn(*in_dev, *self.make_zeros())
        return [
            {
                n: np.asarray(out_arrs[i]).reshape(N_CORES, *self.out_avals[i].shape)[c]
                for i, n in enumerate(self.out_names)
            }
            for c in range(N_CORES)
        ]


_RUNNER = None


def _get_runner():
    global _RUNNER
    if _RUNNER is None:
        _RUNNER = _Runner()
    return _RUNNER


def _make_in_maps(query, key, value, Wq, Wk, Wv, Wo, bq, bk):
    import ml_dtypes

    bf = ml_dtypes.bfloat16
    in_maps = []
    xT = {}
    for b in range(B):
        xT[b] = (
            np.ascontiguousarray(query[b].T).astype(bf),
            np.ascontiguousarray(key[b].T).astype(bf),
            np.ascontiguousarray(value[b].T).astype(bf),
        )
    for c in range(N_CORES):
        b, hg = divmod(c, HG)
        sl = slice(hg * DO, (hg + 1) * DO)
        in_maps.append(
            {
                "xqT": xT[b][0],
                "xkT": xT[b][1],
                "xvT": xT[b][2],
                "wq": np.ascontiguousarray(Wq[:, sl]).astype(bf),
                "wk": np.ascontiguousarray(Wk[:, sl]).astype(bf),
                "wv": np.ascontiguousarray(Wv[:, sl]).astype(bf),
                "wo": np.ascontiguousarray(
                    Wo[sl, :]
                    .reshape(NP, 2, D_HEAD, D_HEAD)
                    .transpose(2, 1, 0, 3)
                    .reshape(D_HEAD, 2 * NP * D_HEAD)
                ).astype(bf),
                "bq": np.ascontiguousarray(bq[sl]),
                "bk": np.ascontiguousarray(bk[sl]),
            }
        )
    return in_maps


def kernel(query, key, value, Wq, bq, Wk, bk, Wv, bv, Wo, bo):
    query = np.ascontiguousarray(np.asarray(query, dtype=np.float32))
    key = np.ascontiguousarray(np.asarray(key, dtype=np.float32))
    value = np.ascontiguousarray(np.asarray(value, dtype=np.float32))
    Wq = np.asarray(Wq, dtype=np.float32)
    Wk = np.asarray(Wk, dtype=np.float32)
    Wv = np.asarray(Wv, dtype=np.float32)
    Wo = np.asarray(Wo, dtype=np.float32)
    bq = np.asarray(bq, dtype=np.float32)
    bk = np.asarray(bk, dtype=np.float32)
    bv = np.asarray(bv, dtype=np.float32)
    bo = np.asarray(bo, dtype=np.float32)

    r = _get_runner()
    in_dev = r.put_inputs(_make_in_maps(query, key, value, Wq, Wk, Wv, Wo, bq, bk))
    results = r.run(in_dev)

    out = np.zeros((B, S, D_HEAD), dtype=np.float32)
    for c in range(N_CORES):
        b = c // HG
        out[b] += results[c]["out"]
    out += bv @ Wo + bo
    return out


def bench(query, key, value, Wq, bq, Wk, bk, Wv, bv, Wo, bo, iters=20):
    """Steady-state per-iteration wall time of the device execution."""
    import time

    r = _get_runner()
    in_dev = r.put_inputs(
        _make_in_maps(
            np.asarray(query, np.float32), np.asarray(key, np.float32),
            np.asarray(value, np.float32), np.asarray(Wq, np.float32),
            np.asarray(Wk, np.float32), np.asarray(Wv, np.float32),
            np.asarray(Wo, np.float32), np.asarray(bq, np.float32),
            np.asarray(bk, np.float32),
        )
    )
    outs = r.fn(*in_dev, *r.make_zeros())
    self_jax = r.jax
    self_jax.block_until_ready(outs)
    zeros = [r.make_zeros() for _ in range(iters)]
    t0 = time.monotonic()
    last = None
    for i in range(iters):
        last = r.fn(*in_dev, *zeros[i])
    self_jax.block_until_ready(last)
    t1 = time.monotonic()
    return (t1 - t0) / iters
